# revision 23
# baseline (speedup 1.0000x reference)
"""AttnPooling Trainium2 kernel: 8-core data-parallel over B*N items.

Per item (b,n): x is (D=128, K=4096) fp32, K contiguous in DRAM.
  mean   = sum_k x[:,k]*m[k] / c           (c = sum m)
  query  = Wq @ mean + bq
  v      = Wk^T query = (Wk^T Wq) mean + Wk^T bq
  s_k    = v . x_k   (+ const that cancels in softmax; bk unused)
  p_k    = m_k exp(s_k/sqrt(D));  out = sum_k p_k x_k / sum_k p_k

v2 structure (vs v1):
 - x is cast fp32->bf16 during the HBM DMA (SWDGE cast) so the mask
   multiply runs as a regular tensor_tensor at DVE 2x bf16 mode.
 - mean accumulation moved off DVE onto the scalar engine
   (activation Copy with accum_out over xm).
 - scores are computed with column-tiled matmuls (tile_position) so the
   4 K-chunks of an item land on PSUM partitions {0,32,64,96}; one exp
   instruction then covers all 4096 scores at free-dim cost 1024.
 - e-broadcast via matmul with a selector stationary that picks rows
   {0,32,64,96}; pooled reduction stays the fused DVE TTR (1x) with
   PSUM in1.
Masking trick as v1: xm = x*m is used for scores and pooled, so masked
positions give exp(0)=1 which is corrected via Z = Zfake - (K - c).
"""

import sys

sys.path.insert(0, "/opt/trn_rl_repo")

import numpy as np
from contextlib import ExitStack

NI = 16  # items per core
D = 128
K = 4096
NCORES = 8
CH = 512  # score chunk width; chunk c lives on PSUM row 32*(c%4)
NCH = K // CH  # 8 chunks
SD = 1.0 / np.sqrt(128.0)  # 1/sqrt(D)

_CACHE = {}


def _build():
    import concourse.bass as bass
    import concourse.tile as tile
    from concourse import bacc, mybir
    from concourse.dve_ops import TENSOR_TENSOR_REDUCE

    dt = mybir.dt
    Alu = mybir.AluOpType
    Act = mybir.ActivationFunctionType

    nc = bacc.Bacc(
        "TRN2", target_bir_lowering=False, debug=False, num_devices=NCORES
    )
    x_d = nc.dram_tensor("x", [NI, D, K], dt.float32, kind="ExternalInput").ap()
    mask_d = nc.dram_tensor("mask", [NI, K], dt.int32, kind="ExternalInput").ap()
    wq_d = nc.dram_tensor("Wq", [D, D], dt.float32, kind="ExternalInput").ap()
    wk_d = nc.dram_tensor("Wk", [D, D], dt.float32, kind="ExternalInput").ap()
    bq_d = nc.dram_tensor("bq", [D, 1], dt.float32, kind="ExternalInput").ap()
    out_d = nc.dram_tensor("out", [D, NI], dt.float32, kind="ExternalOutput").ap()

    with tile.TileContext(nc) as tc, ExitStack() as ctx:
        # SBUF pools
        xbp = ctx.enter_context(tc.tile_pool(name="xbp", bufs=6))
        mbp = ctx.enter_context(tc.tile_pool(name="mbp", bufs=6))
        xmp = ctx.enter_context(tc.tile_pool(name="xmp", bufs=4))
        etp = ctx.enter_context(tc.tile_pool(name="etp", bufs=2))
        junkp = ctx.enter_context(tc.tile_pool(name="junkp", bufs=2))
        accp = ctx.enter_context(tc.tile_pool(name="accp", bufs=6))
        per = ctx.enter_context(tc.tile_pool(name="per", bufs=1))
        # PSUM pools: st4 persistent 2 banks + bc 2x2 banks + tp 2x1 = 8
        stp = ctx.enter_context(tc.tile_pool(name="stp", bufs=1, space="PSUM"))
        bc = ctx.enter_context(tc.tile_pool(name="bc", bufs=2, space="PSUM"))
        tp = ctx.enter_context(tc.tile_pool(name="tp", bufs=2, space="PSUM"))

        # persistent tiles
        wq = per.tile([D, D], dt.float32, tag="wq")
        wk = per.tile([D, D], dt.float32, tag="wk")
        bq = per.tile([D, 1], dt.float32, tag="bq")
        cqk = per.tile([D, D], dt.float32, tag="cqk")
        w0 = per.tile([D, 1], dt.float32, tag="w0")
        ones32f = per.tile([32, D], dt.float32, tag="ones32f")
        sel128 = per.tile([D, D], dt.bfloat16, tag="sel128")
        selcol = per.tile([D, 1], dt.float32, tag="selcol")
        mi32 = per.tile([NI, K], dt.int32, tag="mi32")
        m16 = per.tile([NI, K], dt.bfloat16, tag="m16")
        junk8 = per.tile([D, K], dt.float8e4, tag="junk8")
        cinvrow32 = per.tile([32, NI], dt.float32, tag="cinvrow32")
        cinvb = per.tile([D, NI], dt.float32, tag="cinvb")
        c32 = per.tile([32, 32], dt.float32, tag="c32")
        c32t = per.tile([32, 32], dt.float32, tag="c32t")
        cinvrow = per.tile([1, NI], dt.float32, tag="cinvrow")
        zp4 = per.tile([D, NI], dt.float32, tag="zp4")
        uacc = per.tile([D, NI], dt.float32, tag="uacc")
        vt = per.tile([D, NI], dt.bfloat16, tag="vt")
        praw = per.tile([D, NI], dt.float32, tag="praw")
        zrow = per.tile([1, NI], dt.float32, tag="zrow")
        zinv = per.tile([1, NI], dt.float32, tag="zinv")
        frow32 = per.tile([32, NI], dt.float32, tag="frow32")
        outt = per.tile([D, NI], dt.float32, tag="outt")

        # persistent PSUM score tile (128,1024): chunk c lives at row
        # 32*(c%4), cols [512*(c//4), 512*(c//4+1)); other rows stay 0
        # forever (exp(0)=1, zero-selected by the broadcast matmul).
        st4 = stp.tile([D, 2 * CH], dt.float32, tag="st4")
        nc.vector.memset(st4[:, :], 0.0)

        # ---- setup ----
        nc.sync.dma_start(wq[:, :], wq_d[:, :])
        nc.sync.dma_start(wk[:, :], wk_d[:, :])
        nc.sync.dma_start(bq[:, :], bq_d[:, :])
        nc.sync.dma_start(mi32[:, :], mask_d[:, :])
        nc.vector.memset(ones32f[:, :], 0.0)
        nc.vector.memset(ones32f[0:1, :], 1.0)
        nc.vector.memset(sel128[:, :], 0.0)
        nc.vector.memset(selcol[:, :], 0.0)
        nc.vector.memset(cinvrow32[:, :], 0.0)
        nc.vector.memset(frow32[:, :], 0.0)
        nc.vector.memset(c32[:, :], 0.0)
        for j in range(4):
            nc.vector.memset(sel128[32 * j : 32 * j + 1, :], 1.0)
            nc.vector.memset(selcol[32 * j : 32 * j + 1, :], 1.0)

        # mask -> bf16, with per-item valid counts c into c32[:,0]
        nc.scalar.activation(
            m16[:, :], mi32[:, :], Act.Copy, accum_out=c32[0:NI, 0:1]
        )
        # crow (1,NI) = c^T via 32x32 block transpose
        nc.vector.transpose(c32t[:, :], c32[:, :])
        crow = c32t[0:1, 0:NI]
        nc.vector.reciprocal(cinvrow[:, :], crow)
        # cinvb (D,NI) = per-partition replicated 1/c
        nc.vector.tensor_copy(cinvrow32[0:1, :], cinvrow[:, :])
        cinvb_ps = tp.tile([D, NI], dt.float32, tag="smallps")
        nc.tensor.matmul(
            cinvb_ps[:, :], ones32f[:, :], cinvrow32[:, :], start=True, stop=True
        )
        nc.scalar.copy(cinvb[:, :], cinvb_ps[:, :])

        # CQK = Wq^T Wk ; w0 = Wk^T bq
        cqk_ps = tp.tile([D, D], dt.float32, tag="smallps")
        nc.tensor.matmul(cqk_ps[:, :], wq[:, :], wk[:, :], start=True, stop=True)
        nc.scalar.copy(cqk[:, :], cqk_ps[:, :])
        w0_ps = tp.tile([D, 1], dt.float32, tag="smallps")
        nc.tensor.matmul(w0_ps[:, :], wk[:, :], bq[:, :], start=True, stop=True)
        nc.scalar.copy(w0[:, :], w0_ps[:, :])

        # ---- per-item software pipeline with DMA prefetch ----
        xms = [None] * NI

        xbs = [None] * NI
        mbs = [None] * NI

        def chain_q(i):
            return nc.sync if i % 2 == 0 else nc.gpsimd

        def chain_step(i, s):
            # step s of the mask-broadcast doubling chain for item i:
            # s=0 seeds row 0 from m16; s>=1 doubles rows [0,2^(s-1)) ->
            # [2^(s-1), 2^s). Steps are spread 2-per-iteration across 4
            # iterations so the inter-step DMA waits never block a queue.
            mb = mbs[i]
            q = chain_q(i)
            if s == 0:
                q.dma_start(mb[0:1, :], m16[i : i + 1, :])
            else:
                r = 1 << (s - 1)
                q.dma_start(mb[r : 2 * r, :], mb[0:r, :])

        def dma_phase(i):
            # x cast-load (SWDGE) + first two mask-chain steps; remaining
            # steps are issued by later iterations
            xb = xbp.tile([D, K], dt.bfloat16, tag="xb", name=f"xb_{i}")
            xbs[i] = xb
            nc.gpsimd.dma_start(xb[:, :], x_d[i, :, :])
            mb = mbp.tile([D, K], dt.bfloat16, tag="mb", name=f"mb_{i}")
            mbs[i] = mb
            chain_step(i, 0)
            chain_step(i, 1)

        def tt_phase(i):
            xm = xmp.tile([D, K], dt.bfloat16, tag="xm", name=f"xm_{i}")
            xms[i] = xm
            # xm = xb * mb  (DVE 2x bf16)
            nc.vector.tensor_tensor(xm[:, :], xbs[i][:, :], mbs[i][:, :], op=Alu.mult)

        def actmean_phase(i):
            # u_i = sum_k xm[:,k]  on the scalar engine
            nc.scalar.activation(
                junk8[:, :], xms[i][:, :], Act.Copy,
                accum_out=uacc[:, i : i + 1],
            )

        def v_phase(i):
            # v_i = (CQK^T . u_i) * (1/c_i) + w0   (fp32 matmul, no bf16 hop)
            vps = tp.tile([D, 1], dt.float32, tag="smallps", name=f"vps_{i}")
            nc.tensor.matmul(
                vps[:, :], cqk[:, :], uacc[:, i : i + 1], start=True, stop=True
            )
            nc.vector.tensor_scalar(
                vt[:, i : i + 1],
                vps[:, :],
                cinvb[:, i : i + 1],
                w0[:, 0:1],
                op0=Alu.mult,
                op1=Alu.add,
            )

        def attn_phase(i):
            xm = xms[i]
            # col-tiled scores: chunk c -> st4[c//4] row 32*(c%4). Issued as
            # two waves of 4 matmuls on distinct PE column-groups so they
            # execute concurrently in the array.
            for half in range(2):
                for j in range(4):
                    c = half * 4 + j
                    nc.tensor.matmul(
                        st4[32 * j : 32 * j + 1, half * CH : (half + 1) * CH],
                        vt[:, i : i + 1],
                        xm[:, c * CH : (c + 1) * CH],
                        start=True,
                        stop=True,
                        tile_position=(0, 32 * j),
                    )
            # single exp for all 8 chunks (garbage rows exp(0)=1, zero-sel)
            et = etp.tile([D, 2 * CH], dt.bfloat16, tag="et", name=f"et_{i}")
            nc.scalar.activation(
                et[:, :], st4[:, :], Act.Exp, scale=SD,
                accum_out=zp4[:, i : i + 1],
            )
            # pooled: broadcast e rows in pairs (distinct PE row-groups ->
            # concurrent), then fused TTR over (128, 1024)
            acc_prev = None
            for p in range(NCH // 2):
                bt = bc.tile([D, 2 * CH], dt.float32, tag="bcast", name=f"eb_{i}_{p}")
                for h in range(2):
                    c = 2 * p + h
                    half = c // 4
                    j = c % 4
                    nc.tensor.matmul(
                        bt[:, h * CH : (h + 1) * CH],
                        sel128[32 * j : 32 * j + 32, :],
                        et[32 * j : 32 * j + 32, half * CH : (half + 1) * CH],
                        start=True,
                        stop=True,
                        tile_position=(32 * j, 0),
                    )
                jt = junkp.tile([D, 2 * CH], dt.bfloat16, tag="jtt", name=f"jtt_{i}_{p}")
                acc = (
                    praw[:, i : i + 1]
                    if p == NCH // 2 - 1
                    else accp.tile([D, 1], dt.float32, tag="pacc", name=f"pa_{i}_{p}")
                )
                nc.vector._custom_dve(
                    TENSOR_TENSOR_REDUCE,
                    out=jt[:, :],
                    in0=xm[:, 2 * p * CH : 2 * (p + 1) * CH],
                    in1=bt[:, :],
                    s0=(0.0 if acc_prev is None else acc_prev[:, 0:1]),
                    s1=1.0,
                    accum_out=acc[:, 0:1],
                )
                acc_prev = acc

        # per-iteration engine-queue order chosen to avoid head-of-line
        # blocking: DVE sees [TT(j), TTR(a)x4, ts(w)], Scalar sees
        # [exp(a)x2, mean(j)] -- every op's deps are >=1 iteration old.
        for i in range(NI + 7):
            if i < NI:
                dma_phase(i)
            for back, steps in ((1, (2, 3)), (2, (4, 5)), (3, (6, 7))):
                b = i - back
                if 0 <= b < NI:
                    for s in steps:
                        chain_step(b, s)
            j = i - 4
            if 0 <= j < NI:
                tt_phase(j)
            w = i - 5
            if 0 <= w < NI:
                v_phase(w)
            a = i - 6
            if 0 <= a < NI:
                attn_phase(a)
            if 0 <= j < NI:
                actmean_phase(j)

        # ---- finalize: Zfake_i = sum over rows {0,32,64,96} of zp4 ----
        zf_ps = tp.tile([1, NI], dt.float32, tag="smallps")
        nc.tensor.matmul(
            zf_ps[:, :], selcol[:, :], zp4[:, :], start=True, stop=True
        )
        # zrow = (zfake + c) - K ;  out = praw / Z
        nc.vector.tensor_tensor(zrow[:, :], zf_ps[:, :], crow, op=Alu.add)
        nc.vector.tensor_scalar(
            zrow[:, :], zrow[:, :], -float(K), None, op0=Alu.add
        )
        nc.vector.reciprocal(zinv[:, :], zrow[:, :])
        nc.vector.tensor_copy(frow32[0:1, :], zinv[:, :])
        fb = tp.tile([D, NI], dt.float32, tag="smallps")
        nc.tensor.matmul(
            fb[:, :], ones32f[:, :], frow32[:, :], start=True, stop=True
        )
        nc.vector.tensor_tensor(outt[:, :], praw[:, :], fb[:, :], op=Alu.mult)
        nc.sync.dma_start(out_d[:, :], outt[:, :])

    nc.compile()
    return nc


def _get_nc():
    if "nc" not in _CACHE:
        _CACHE["nc"] = _build()
    return _CACHE["nc"]


def _make_in_maps(inputs):
    x, mask = inputs["x"], inputs["mask"]
    B, N, d, H, W = x.shape
    xr = np.ascontiguousarray(x.reshape(B * N, d, H * W).astype(np.float32))
    mr = np.ascontiguousarray(mask.reshape(B * N, H * W).astype(np.int32))
    bq2 = np.ascontiguousarray(inputs["bq"].reshape(d, 1).astype(np.float32))
    wqc = np.ascontiguousarray(inputs["Wq"].astype(np.float32))
    wkc = np.ascontiguousarray(inputs["Wk"].astype(np.float32))
    in_maps = []
    for c in range(NCORES):
        s = slice(c * NI, (c + 1) * NI)
        in_maps.append(
            {
                "x": np.ascontiguousarray(xr[s]),
                "mask": np.ascontiguousarray(mr[s]),
                "Wq": wqc,
                "Wk": wkc,
                "bq": bq2,
            }
        )
    return in_maps


def _gather(results, inputs):
    B, N, d = inputs["x"].shape[:3]
    parts = [np.asarray(results[c]["out"]).T for c in range(NCORES)]
    return np.concatenate(parts, axis=0).reshape(B, N, d).astype(np.float32)


def kernel(x, mask, Wq, bq, Wk, bk):
    from concourse.bass_utils import run_bass_kernel_spmd

    nc = _get_nc()
    inputs = {"x": x, "mask": mask, "Wq": Wq, "bq": bq, "Wk": Wk, "bk": bk}
    in_maps = _make_in_maps(inputs)
    res = run_bass_kernel_spmd(nc, in_maps, core_ids=list(range(NCORES)))
    return _gather(res.results, inputs)


# revision 25
# speedup vs baseline: 1.3094x; 1.3094x over previous
"""AttnPooling Trainium2 kernel: 8-core data-parallel over B*N items.

Per item (b,n): x is (D=128, K=4096) fp32, K contiguous in DRAM.
  mean   = sum_k x[:,k]*m[k] / c           (c = sum m)
  query  = Wq @ mean + bq
  v      = Wk^T query = (Wk^T Wq) mean + Wk^T bq
  s_k    = v . x_k   (+ const that cancels in softmax; bk unused)
  p_k    = m_k exp(s_k/sqrt(D));  out = sum_k p_k x_k / sum_k p_k

v2 structure (vs v1):
 - x is cast fp32->bf16 during the HBM DMA (SWDGE cast) so the mask
   multiply runs as a regular tensor_tensor at DVE 2x bf16 mode.
 - mean accumulation moved off DVE onto the scalar engine
   (activation Copy with accum_out over xm).
 - scores are computed with column-tiled matmuls (tile_position) so the
   4 K-chunks of an item land on PSUM partitions {0,32,64,96}; one exp
   instruction then covers all 4096 scores at free-dim cost 1024.
 - e-broadcast via matmul with a selector stationary that picks rows
   {0,32,64,96}; pooled reduction stays the fused DVE TTR (1x) with
   PSUM in1.
Masking trick as v1: xm = x*m is used for scores and pooled, so masked
positions give exp(0)=1 which is corrected via Z = Zfake - (K - c).
"""

import sys

sys.path.insert(0, "/opt/trn_rl_repo")

import numpy as np
from contextlib import ExitStack

NI = 16  # items per core
D = 128
K = 4096
NCORES = 8
CH = 512  # score chunk width; chunk c lives on PSUM row 32*(c%4)
NCH = K // CH  # 8 chunks
SD = 1.0 / np.sqrt(128.0)  # 1/sqrt(D)

_CACHE = {}


def _build():
    import concourse.bass as bass
    import concourse.tile as tile
    from concourse import bacc, mybir
    from concourse.dve_ops import TENSOR_TENSOR_REDUCE

    dt = mybir.dt
    Alu = mybir.AluOpType
    Act = mybir.ActivationFunctionType

    nc = bacc.Bacc(
        "TRN2", target_bir_lowering=False, debug=False, num_devices=NCORES
    )
    x_d = nc.dram_tensor("x", [NI, D, K], dt.float32, kind="ExternalInput").ap()
    mask_d = nc.dram_tensor("mask", [NI, K], dt.int32, kind="ExternalInput").ap()
    wq_d = nc.dram_tensor("Wq", [D, D], dt.float32, kind="ExternalInput").ap()
    wk_d = nc.dram_tensor("Wk", [D, D], dt.float32, kind="ExternalInput").ap()
    bq_d = nc.dram_tensor("bq", [D, 1], dt.float32, kind="ExternalInput").ap()
    out_d = nc.dram_tensor("out", [D, NI], dt.float32, kind="ExternalOutput").ap()

    with tile.TileContext(nc) as tc, ExitStack() as ctx:
        # SBUF pools
        xp = ctx.enter_context(tc.tile_pool(name="xp", bufs=5))
        xmp = ctx.enter_context(tc.tile_pool(name="xmp", bufs=4))
        etp = ctx.enter_context(tc.tile_pool(name="etp", bufs=2))
        junkp = ctx.enter_context(tc.tile_pool(name="junkp", bufs=2))
        accp = ctx.enter_context(tc.tile_pool(name="accp", bufs=6))
        per = ctx.enter_context(tc.tile_pool(name="per", bufs=1))
        # PSUM pools: st4 persistent 2 banks + bc 2x2 banks + tp 2x1 = 8
        stp = ctx.enter_context(tc.tile_pool(name="stp", bufs=1, space="PSUM"))
        bc = ctx.enter_context(tc.tile_pool(name="bc", bufs=2, space="PSUM"))
        tp = ctx.enter_context(tc.tile_pool(name="tp", bufs=2, space="PSUM"))

        # persistent tiles
        wq = per.tile([D, D], dt.float32, tag="wq")
        wk = per.tile([D, D], dt.float32, tag="wk")
        bq = per.tile([D, 1], dt.float32, tag="bq")
        cqk = per.tile([D, D], dt.float32, tag="cqk")
        w0 = per.tile([D, 1], dt.float32, tag="w0")
        ones32f = per.tile([32, D], dt.float32, tag="ones32f")
        sel128 = per.tile([D, D], dt.bfloat16, tag="sel128")
        selcol = per.tile([D, 1], dt.float32, tag="selcol")
        mi32 = per.tile([NI, K], dt.int32, tag="mi32")
        m16 = per.tile([NI, K], dt.bfloat16, tag="m16")
        msel = per.tile([NI, NI * D], dt.bfloat16, tag="msel")
        cinvrow32 = per.tile([32, NI], dt.float32, tag="cinvrow32")
        cinvb = per.tile([D, NI], dt.float32, tag="cinvb")
        c32 = per.tile([32, 32], dt.float32, tag="c32")
        c32t = per.tile([32, 32], dt.float32, tag="c32t")
        cinvrow = per.tile([1, NI], dt.float32, tag="cinvrow")
        zp4 = per.tile([D, NI], dt.float32, tag="zp4")
        uacc = per.tile([D, NI], dt.float32, tag="uacc")
        vt = per.tile([D, NI], dt.bfloat16, tag="vt")
        praw = per.tile([D, NI], dt.float32, tag="praw")
        zrow = per.tile([1, NI], dt.float32, tag="zrow")
        zinv = per.tile([1, NI], dt.float32, tag="zinv")
        frow32 = per.tile([32, NI], dt.float32, tag="frow32")
        outt = per.tile([D, NI], dt.float32, tag="outt")

        # persistent PSUM score tile (128,1024): chunk c lives at row
        # 32*(c%4), cols [512*(c//4), 512*(c//4+1)); other rows stay 0
        # forever (exp(0)=1, zero-selected by the broadcast matmul).
        st4 = stp.tile([D, 2 * CH], dt.float32, tag="st4")
        nc.vector.memset(st4[:, :], 0.0)

        # ---- setup ----
        nc.sync.dma_start(wq[:, :], wq_d[:, :])
        nc.sync.dma_start(wk[:, :], wk_d[:, :])
        nc.sync.dma_start(bq[:, :], bq_d[:, :])
        nc.sync.dma_start(mi32[:, :], mask_d[:, :])
        nc.vector.memset(ones32f[:, :], 0.0)
        nc.vector.memset(ones32f[0:1, :], 1.0)
        nc.vector.memset(sel128[:, :], 0.0)
        nc.vector.memset(selcol[:, :], 0.0)
        nc.vector.memset(cinvrow32[:, :], 0.0)
        nc.vector.memset(frow32[:, :], 0.0)
        nc.vector.memset(c32[:, :], 0.0)
        for j in range(4):
            nc.vector.memset(sel128[32 * j : 32 * j + 1, :], 1.0)
            nc.vector.memset(selcol[32 * j : 32 * j + 1, :], 1.0)
        nc.vector.memset(msel[0:NI, :], 0.0)
        for i in range(NI):
            # engine ops can't address partition i directly; DMA can
            nc.sync.dma_start(msel[i : i + 1, i * D : (i + 1) * D], sel128[0:1, :])

        # mask -> bf16, with per-item valid counts c into c32[:,0]
        nc.scalar.activation(
            m16[:, :], mi32[:, :], Act.Copy, accum_out=c32[0:NI, 0:1]
        )
        # crow (1,NI) = c^T via 32x32 block transpose
        nc.vector.transpose(c32t[:, :], c32[:, :])
        crow = c32t[0:1, 0:NI]
        nc.vector.reciprocal(cinvrow[:, :], crow)
        # cinvb (D,NI) = per-partition replicated 1/c
        nc.vector.tensor_copy(cinvrow32[0:1, :], cinvrow[:, :])
        cinvb_ps = tp.tile([D, NI], dt.float32, tag="smallps")
        nc.tensor.matmul(
            cinvb_ps[:, :], ones32f[:, :], cinvrow32[:, :], start=True, stop=True
        )
        nc.scalar.copy(cinvb[:, :], cinvb_ps[:, :])

        # CQK = Wq^T Wk ; w0 = Wk^T bq
        cqk_ps = tp.tile([D, D], dt.float32, tag="smallps")
        nc.tensor.matmul(cqk_ps[:, :], wq[:, :], wk[:, :], start=True, stop=True)
        nc.scalar.copy(cqk[:, :], cqk_ps[:, :])
        w0_ps = tp.tile([D, 1], dt.float32, tag="smallps")
        nc.tensor.matmul(w0_ps[:, :], wk[:, :], bq[:, :], start=True, stop=True)
        nc.scalar.copy(w0[:, :], w0_ps[:, :])

        # ---- per-item software pipeline with DMA prefetch ----
        xms = [None] * NI

        xbs = [None] * NI
        mbs = [None] * NI

        def dma_phase(i):
            # plain fp32 x load (HWDGE)
            xt = xp.tile([D, K], dt.float32, tag="xt", name=f"xt_{i}")
            xbs[i] = xt
            nc.sync.dma_start(xt[:, :], x_d[i, :, :])

        def mean_phase(i):
            # mask rows broadcast on the PE (selector matmul) feeding the
            # fused TTR: xm = x*m (bf16 out) with accum -> u_i. No DMA.
            xt = xbs[i]
            xm = xmp.tile([D, K], dt.bfloat16, tag="xm", name=f"xm_{i}")
            xms[i] = xm
            acc_prev = None
            for p in range(NCH // 2):
                mps = bc.tile([D, 2 * CH], dt.float32, tag="bcast", name=f"mp_{i}_{p}")
                for h in range(2):
                    c = 2 * p + h
                    nc.tensor.matmul(
                        mps[:, h * CH : (h + 1) * CH],
                        msel[0:NI, i * D : (i + 1) * D],
                        m16[0:NI, c * CH : (c + 1) * CH],
                        start=True,
                        stop=True,
                    )
                acc = (
                    uacc[:, i : i + 1]
                    if p == NCH // 2 - 1
                    else accp.tile([D, 1], dt.float32, tag="uacc", name=f"ua_{i}_{p}")
                )
                nc.vector._custom_dve(
                    TENSOR_TENSOR_REDUCE,
                    out=xm[:, 2 * p * CH : 2 * (p + 1) * CH],
                    in0=xt[:, 2 * p * CH : 2 * (p + 1) * CH],
                    in1=mps[:, :],
                    s0=(0.0 if acc_prev is None else acc_prev[:, 0:1]),
                    s1=1.0,
                    accum_out=acc[:, 0:1],
                )
                acc_prev = acc

        def v_phase(i):
            # v_i = (CQK^T . u_i) * (1/c_i) + w0   (fp32 matmul, no bf16 hop)
            vps = tp.tile([D, 1], dt.float32, tag="smallps", name=f"vps_{i}")
            nc.tensor.matmul(
                vps[:, :], cqk[:, :], uacc[:, i : i + 1], start=True, stop=True
            )
            nc.vector.tensor_scalar(
                vt[:, i : i + 1],
                vps[:, :],
                cinvb[:, i : i + 1],
                w0[:, 0:1],
                op0=Alu.mult,
                op1=Alu.add,
            )

        def attn_phase(i):
            xm = xms[i]
            # col-tiled scores: chunk c -> st4[c//4] row 32*(c%4). Issued as
            # two waves of 4 matmuls on distinct PE column-groups so they
            # execute concurrently in the array.
            for half in range(2):
                for j in range(4):
                    c = half * 4 + j
                    nc.tensor.matmul(
                        st4[32 * j : 32 * j + 1, half * CH : (half + 1) * CH],
                        vt[:, i : i + 1],
                        xm[:, c * CH : (c + 1) * CH],
                        start=True,
                        stop=True,
                        tile_position=(0, 32 * j),
                    )
            # single exp for all 8 chunks (garbage rows exp(0)=1, zero-sel)
            et = etp.tile([D, 2 * CH], dt.bfloat16, tag="et", name=f"et_{i}")
            nc.scalar.activation(
                et[:, :], st4[:, :], Act.Exp, scale=SD,
                accum_out=zp4[:, i : i + 1],
            )
            # pooled: broadcast e rows in pairs (distinct PE row-groups ->
            # concurrent), then fused TTR over (128, 1024)
            acc_prev = None
            for p in range(NCH // 2):
                bt = bc.tile([D, 2 * CH], dt.float32, tag="bcast", name=f"eb_{i}_{p}")
                for h in range(2):
                    c = 2 * p + h
                    half = c // 4
                    j = c % 4
                    nc.tensor.matmul(
                        bt[:, h * CH : (h + 1) * CH],
                        sel128[32 * j : 32 * j + 32, :],
                        et[32 * j : 32 * j + 32, half * CH : (half + 1) * CH],
                        start=True,
                        stop=True,
                        tile_position=(32 * j, 0),
                    )
                jt = junkp.tile([D, 2 * CH], dt.bfloat16, tag="jtt", name=f"jtt_{i}_{p}")
                acc = (
                    praw[:, i : i + 1]
                    if p == NCH // 2 - 1
                    else accp.tile([D, 1], dt.float32, tag="pacc", name=f"pa_{i}_{p}")
                )
                nc.vector._custom_dve(
                    TENSOR_TENSOR_REDUCE,
                    out=jt[:, :],
                    in0=xm[:, 2 * p * CH : 2 * (p + 1) * CH],
                    in1=bt[:, :],
                    s0=(0.0 if acc_prev is None else acc_prev[:, 0:1]),
                    s1=1.0,
                    accum_out=acc[:, 0:1],
                )
                acc_prev = acc

        # per-iteration engine-queue order chosen to avoid head-of-line
        # blocking: DVE sees [TT(j), TTR(a)x4, ts(w)], Scalar sees
        # [exp(a)x2, mean(j)] -- every op's deps are >=1 iteration old.
        for i in range(NI + 7):
            if i < NI:
                dma_phase(i)
            j = i - 4
            if 0 <= j < NI:
                mean_phase(j)
            w = i - 5
            if 0 <= w < NI:
                v_phase(w)
            a = i - 6
            if 0 <= a < NI:
                attn_phase(a)

        # ---- finalize: Zfake_i = sum over rows {0,32,64,96} of zp4 ----
        zf_ps = tp.tile([1, NI], dt.float32, tag="smallps")
        nc.tensor.matmul(
            zf_ps[:, :], selcol[:, :], zp4[:, :], start=True, stop=True
        )
        # zrow = (zfake + c) - K ;  out = praw / Z
        nc.vector.tensor_tensor(zrow[:, :], zf_ps[:, :], crow, op=Alu.add)
        nc.vector.tensor_scalar(
            zrow[:, :], zrow[:, :], -float(K), None, op0=Alu.add
        )
        nc.vector.reciprocal(zinv[:, :], zrow[:, :])
        nc.vector.tensor_copy(frow32[0:1, :], zinv[:, :])
        fb = tp.tile([D, NI], dt.float32, tag="smallps")
        nc.tensor.matmul(
            fb[:, :], ones32f[:, :], frow32[:, :], start=True, stop=True
        )
        nc.vector.tensor_tensor(outt[:, :], praw[:, :], fb[:, :], op=Alu.mult)
        nc.sync.dma_start(out_d[:, :], outt[:, :])

    nc.compile()
    return nc


def _get_nc():
    if "nc" not in _CACHE:
        _CACHE["nc"] = _build()
    return _CACHE["nc"]


def _make_in_maps(inputs):
    x, mask = inputs["x"], inputs["mask"]
    B, N, d, H, W = x.shape
    xr = np.ascontiguousarray(x.reshape(B * N, d, H * W).astype(np.float32))
    mr = np.ascontiguousarray(mask.reshape(B * N, H * W).astype(np.int32))
    bq2 = np.ascontiguousarray(inputs["bq"].reshape(d, 1).astype(np.float32))
    wqc = np.ascontiguousarray(inputs["Wq"].astype(np.float32))
    wkc = np.ascontiguousarray(inputs["Wk"].astype(np.float32))
    in_maps = []
    for c in range(NCORES):
        s = slice(c * NI, (c + 1) * NI)
        in_maps.append(
            {
                "x": np.ascontiguousarray(xr[s]),
                "mask": np.ascontiguousarray(mr[s]),
                "Wq": wqc,
                "Wk": wkc,
                "bq": bq2,
            }
        )
    return in_maps


def _gather(results, inputs):
    B, N, d = inputs["x"].shape[:3]
    parts = [np.asarray(results[c]["out"]).T for c in range(NCORES)]
    return np.concatenate(parts, axis=0).reshape(B, N, d).astype(np.float32)


def kernel(x, mask, Wq, bq, Wk, bk):
    from concourse.bass_utils import run_bass_kernel_spmd

    nc = _get_nc()
    inputs = {"x": x, "mask": mask, "Wq": Wq, "bq": bq, "Wk": Wk, "bk": bk}
    in_maps = _make_in_maps(inputs)
    res = run_bass_kernel_spmd(nc, in_maps, core_ids=list(range(NCORES)))
    return _gather(res.results, inputs)
